# revision 12
# baseline (speedup 1.0000x reference)
"""Trainium2 Bass kernel for nn_AttentionResBlock (windowed causal attention +
sigmoid*tanh gating + two 1x1 convs), SPMD over 8 NeuronCores.

Sharding: data-parallel over (batch, sequence-half): core i handles batch i//2,
rows [h*2048, (h+1)*2048) with h = i%2, plus a 512-row halo (previous window;
zeros + mask flag for h==0). No cross-core communication.

Layout strategy (v2): the host pre-computes BOTH layouts of x the PE needs —
xt (c-major, [128, 2cc, t]) for the QK/AV lhsT/rhs operands and xn (t-major,
[128, s, c+2] with a baked [1, 0] tail whose ones-column yields the softmax
denominator inside the AV matmul) — so the device does no input transposes at
all and every DMA lands with >=2KB contiguous per-partition segments.

Per-core pipeline (window = 512 queries, kv = 1024 keys):
  scoresT[j,q] = kvT^T @ qT        (PE bf16; jc5/jc6/jc7 trimmed to their
      causally-valid q ranges; garbage in merged tiles is exp'd then zeroed
      by the same affine_select that applies the causal mask)
  expT = exp(scale*scoresT)        (ACT, PSUM->SBUF, bf16)
  o_unnorm[q, c+2] = sum_j expT^T @ [v | 1 | 0]   (PE)
  o = o_unnorm * recip(denom)      (DVE per-partition scalar, PSUM drain)
  oT via PE transpose; gating u = tanh(a)*(1+tanh(a/2)) reads the transpose
      PSUM directly (ACT tanh x2 + one fused DVE scalar_tensor_tensor); the
      0.5 of sigmoid(a) = (1+tanh(a/2))/2 is folded into the weights.
  projT[d, t] = wc^T @ uT          (PE; res/skip fused along d = 4 chunks of
      128; bias is now PER-PARTITION so the PSUM drain is a single DVE
      tensor_scalar add, bf16 out; the host un-transposes the [d, t] output)

Schedule: software pipeline with a TWO-round lag on the output path: round r
runs QK/exp/AV/transpose for window r, the gating for window r-1 (emitted
after round r's exps so they feed AV first), and the projections for window
r-2 (whose inputs are a full round old — they interleave into round r's
exp-latency stalls without ever blocking the in-order PE queue). Junk
matmuls warm the HAM clock-gate during the initial DMA window. PSUM: 4 score
slots + 2 shared AV/proj slots + 2 transpose slots = exactly 8 banks.
"""

import numpy as np

B, T, C = 4, 4096, 256
W = 512                # attention window
TCH = T // 2           # rows per core
TH = TCH + W           # with halo
NWIN = TCH // W        # windows per core (4)
NBLK = TH // W         # 512-row blocks (5)
NCORES = 8

_CACHE = {}


def _build_program(qk_dtype_name="bfloat16"):
    import concourse.bacc as bacc
    import concourse.bass as bass
    import concourse.mybir as mybir
    import concourse.tile as tile
    from concourse.masks import make_identity

    f32 = mybir.dt.float32
    qdt = getattr(mybir.dt, qk_dtype_name)
    ts = bass.ts

    nc = bacc.Bacc("TRN2", target_bir_lowering=False, debug=False)

    xt_d = nc.dram_tensor("xt", [128, NBLK, 2, W], qdt, kind="ExternalInput").ap()
    xn_d = nc.dram_tensor("xn", [128, NBLK, 4, C + 2], qdt, kind="ExternalInput").ap()
    wc_d = nc.dram_tensor("wc", [128, 2, 2 * C], qdt, kind="ExternalInput").ap()
    bb_d = nc.dram_tensor("bb", [128, 4], f32, kind="ExternalInput").ap()
    hflag = nc.dram_tensor("hflag", [128, 1], f32, kind="ExternalInput").ap()
    out_d = nc.dram_tensor("out", [128, 4, TCH], qdt, kind="ExternalOutput").ap()

    Exp = mybir.ActivationFunctionType.Exp
    Tanh = mybir.ActivationFunctionType.Tanh
    Add = mybir.AluOpType.add
    Mult = mybir.AluOpType.mult

    with tile.TileContext(nc) as tc:
        with (
            tc.tile_pool(name="singles", bufs=1) as singles,
            tc.tile_pool(name="xt", bufs=1) as xt_pool,
            tc.tile_pool(name="xn", bufs=1) as xn_pool,
            tc.tile_pool(name="ex", bufs=9) as ex_pool,
            tc.tile_pool(name="on", bufs=6) as on_pool,
            tc.tile_pool(name="g", bufs=4) as g_pool,
            tc.tile_pool(name="u", bufs=2) as u_pool,
            tc.tile_pool(name="outs", bufs=2) as out_pool,
            tc.tile_pool(name="small", bufs=8) as small,
            tc.tile_pool(name="psc", bufs=2, space="PSUM") as sc_pool,
            tc.tile_pool(name="pwork", bufs=2, space="PSUM") as work_pool,
            tc.tile_pool(name="pt", bufs=2, space="PSUM") as pt_pool,
        ):
            # ---- input DMAs, all on the sync HWDGE queue, ordered by when
            # consumers need them (xt0/xt1 gate the first QK). Scalar queue
            # stays DMA-free so the ACT pipeline is never stalled behind a
            # descriptor-gen instruction.
            xtb = [None] * NBLK
            xnb = [None] * NBLK
            hf_sb = singles.tile([128, 1], f32)
            wc_sb = singles.tile([128, 2, 2 * C], qdt)
            bb_sb = singles.tile([128, 4], f32)

            # combined loads: one DMA for blocks 0-1 (gates the first QK/AV)
            # and one for blocks 2-4 (consumed a full round later), so the
            # first window's data lands behind a single issue+completion.
            xt01 = xt_pool.tile([128, 2, 2, W], qdt, tag="xt01")
            nc.sync.dma_start(out=xt01, in_=xt_d[:, 0:2, :, :])
            xn01 = xn_pool.tile([128, 2, 4, C + 2], qdt, tag="xn01")
            nc.sync.dma_start(out=xn01, in_=xn_d[:, 0:2, :, :])
            nc.sync.dma_start(out=wc_sb, in_=wc_d)
            nc.sync.dma_start(out=hf_sb, in_=hflag)
            nc.sync.dma_start(out=bb_sb, in_=bb_d)
            xt234 = xt_pool.tile([128, 3, 2, W], qdt, tag="xt234")
            nc.sync.dma_start(out=xt234, in_=xt_d[:, 2:5, :, :])
            xn234 = xn_pool.tile([128, 3, 4, C + 2], qdt, tag="xn234")
            nc.sync.dma_start(out=xn234, in_=xn_d[:, 2:5, :, :])
            for blk in range(2):
                xtb[blk] = xt01[:, blk]
                xnb[blk] = xn01[:, blk]
            for blk in range(2, 5):
                xtb[blk] = xt234[:, blk - 2]
                xnb[blk] = xn234[:, blk - 2]

            # touch exp once so the ACT table set loads during the DMA shadow
            actwarm = small.tile([128, 1], f32, tag="rc")
            nc.vector.memset(actwarm, 0.0)
            nc.scalar.activation(out=actwarm, in_=actwarm, func=Exp)

            # HAM warmup: junk matmuls keep the PE activity window non-idle
            # from ~immediately after the NEFF barrier until the first real
            # QK, so the 4/8->8/8 clock-gate ramp happens during the DMA
            # shadow instead of eating the first windows' matmuls. Sized to
            # undershoot the DMA window (a short PE gap is harmless; junk
            # overrunning it would stall the first QK on the in-order queue).
            junk = singles.tile([128, 2 * C], qdt)
            nc.vector.memset(junk, 0.0)
            identf = singles.tile([128, 128], f32)
            make_identity(nc, identf)
            ident = singles.tile([128, 128], qdt)
            nc.vector.tensor_copy(ident, identf)
            for i in range(8):
                pwarm = work_pool.tile([128, 384], f32, tag="work")
                nc.tensor.matmul(
                    pwarm, junk[:, 0:128], junk[:, 0:384], start=True, stop=True
                )

            # ---------------- per-round emission helpers ----------------
            def qk_exp_pair(w, pa):
                """QK matmuls + one merged exp for a pair of 128-row j-chunks
                (jc = 2*pa, 2*pa+1). The diagonal chunks only compute their
                causally-valid q ranges; the unwritten PSUM region is
                pending-zeroed by the group start, exp'd, then zeroed again by
                the affine_select that applies the causal mask."""
                qt = xtb[w + 1]
                qw = 256 if pa == 3 else W
                q_lo = W - qw
                psc = sc_pool.tile([128, 2, qw], f32, tag="sc", name=f"sc{pa}")
                for i, jc in enumerate((2 * pa, 2 * pa + 1)):
                    # each 128-row sub-chunk is its own accumulation group:
                    # a single group's start pending-zero only covers one
                    # PSUM bank, and this tile spans two
                    kvt = xtb[w + jc // 4]
                    for cc in range(2):
                        nc.tensor.matmul(
                            psc[:, i, :],
                            kvt[:, cc, ts(jc % 4, 128)],
                            qt[:, cc, q_lo:W],
                            start=(cc == 0),
                            stop=(cc == 1),
                        )
                ex = ex_pool.tile([128, 2, qw], qdt, tag="ex", name=f"ex{pa}")
                nc.scalar.activation(out=ex, in_=psc, func=Exp, scale=0.0625)
                if pa >= 2:
                    # causal mask: valid iff q >= j - 512; for sub-tile i the
                    # chunk base j0 = pa*256 + i*128, cols are q - (W - qw)
                    nc.gpsimd.affine_select(
                        out=ex,
                        in_=ex,
                        compare_op=mybir.AluOpType.is_ge,
                        fill=0.0,
                        base=(W - qw) + 512 - pa * 256,
                        channel_multiplier=-1,
                        pattern=[[-128, 2], [1, qw]],
                    )
                return ex

            def av_qb(w, qb, expts):
                """one 128-query block of AV (+denominator) + normalize."""
                jcs = list(range(min(qb + 5, 8)))
                pav = work_pool.tile([128, C + 2], f32, tag="work")
                for k, jc in enumerate(jcs):
                    ap, q_lo = expts[jc]
                    nc.tensor.matmul(
                        pav,
                        ap[:, qb * 128 - q_lo : qb * 128 - q_lo + 128],
                        xnb[w + jc // 4][:, jc % 4, :],
                        start=(k == 0),
                        stop=(k == len(jcs) - 1),
                    )
                rc = small.tile([128, 1], f32, tag="rc")
                nc.vector.reciprocal(rc, pav[:, C : C + 1])
                on = on_pool.tile([128, C], qdt, tag="on")
                nc.vector.tensor_scalar_mul(on, pav[:, 0:C], rc)
                return on

            def proj_chunk(wp, d, u, outw, last=False):
                """one 128-channel output chunk of the fused res|skip
                projection, transposed: psp[d, t] = wc_d^T @ uT; drain is a
                per-partition-bias DVE add, bf16 out."""
                psp = work_pool.tile([128, W], f32, tag="work")
                for cc in range(2):
                    nc.tensor.matmul(
                        psp,
                        wc_sb[:, cc, ts(d, 128)],
                        u[:, cc, :],
                        start=(cc == 0),
                        stop=(cc == 1),
                    )
                nc.vector.tensor_scalar_add(outw[:, d, :], psp, bb_sb[:, d : d + 1])
                if last:
                    nc.sync.dma_start(
                        out=out_d[:, d, ts(wp, W)], in_=outw[:, d, :]
                    )

            pts = {}   # window -> oT transpose PSUM tile
            us = {}    # window -> gated uT SBUF tile
            outws = {} # window -> output staging tile

            for r in range(NWIN + 1):
                w = r if r < NWIN else None
                wg = r - 1  # window whose gating runs this round
                wp = r - 2  # window whose projections run this round
                if wp >= 0:
                    outws[wp] = out_pool.tile([128, 4, W], qdt, tag="outs", name=f"outw{wp}")

                if w is not None:
                    # QK pair groups with wp's projections interleaved at the
                    # points where the PE would otherwise wait on exp slots
                    ex01 = qk_exp_pair(w, 0)
                    if wp >= 0:
                        proj_chunk(wp, 0, us[wp], outws[wp])
                    ex23 = qk_exp_pair(w, 1)
                    if wp >= 0:
                        proj_chunk(wp, 1, us[wp], outws[wp])
                    ex45 = qk_exp_pair(w, 2)
                    if wp >= 0:
                        proj_chunk(wp, 2, us[wp], outws[wp])
                    ex67 = qk_exp_pair(w, 3)
                    expts = [
                        (ex01[:, 0, :], 0),
                        (ex01[:, 1, :], 0),
                        (ex23[:, 0, :], 0),
                        (ex23[:, 1, :], 0),
                        (ex45[:, 0, :], 0),
                        (ex45[:, 1, :], 0),
                        (ex67[:, 0, :], 256),
                        (ex67[:, 1, :], 256),
                    ]
                    if w == 0:
                        # halo validity flag (1.0 = real halo, 0 = first win)
                        nc.vector.tensor_scalar_mul(ex01, ex01, hf_sb)
                        nc.vector.tensor_scalar_mul(ex23, ex23, hf_sb)
                    ons = [av_qb(w, 0, expts)]
                    if wp >= 0:
                        proj_chunk(wp, 3, us[wp], outws[wp])
                    for qb in range(1, 4):
                        ons.append(av_qb(w, qb, expts))
                    pt4 = pt_pool.tile([128, 2, W], qdt, tag="pt")
                    for qb in range(4):
                        for cc in range(2):
                            nc.tensor.transpose(
                                pt4[:, cc, ts(qb, 128)],
                                ons[qb][:, ts(cc, 128)],
                                ident,
                            )
                    pts[w] = pt4
                elif wp >= 0:
                    # epilogue round: wp = NWIN-2 projections
                    for d in range(4):
                        proj_chunk(wp, d, us[wp], outws[wp])
                    nc.sync.dma_start(out=out_d[:, :, ts(wp, W)], in_=outws[wp])

                if wg >= 0 and wg < NWIN - 1:
                    # gating for window wg: emitted after round r's exps so
                    # those win the ACT queue; u = tanh(a) * (1 + tanh(a/2)).
                    # The combine runs on the (otherwise idle) gpsimd engine —
                    # the two-round lag gives it a full round of slack.
                    pt4 = pts.pop(wg)
                    ta = g_pool.tile([128, 2, W], qdt, tag="g")
                    th2 = g_pool.tile([128, 2, W], qdt, tag="g")
                    nc.scalar.activation(out=ta, in_=pt4, func=Tanh)
                    nc.scalar.activation(out=th2, in_=pt4, func=Tanh, scale=0.5)
                    uu = u_pool.tile([128, 2, W], qdt, tag="u")
                    nc.vector.scalar_tensor_tensor(
                        out=uu, in0=th2, scalar=1.0, in1=ta, op0=Add, op1=Mult
                    )
                    us[wg] = uu

                if w is None and wp >= 0:
                    # epilogue: final window's gating + projections pipelined
                    # per contraction half (cc) so the PE never idles long
                    # enough to trip the HAM MID re-throttle, with the cc0
                    # matmuls starting while cc1's tanh still runs.
                    wl = NWIN - 1
                    pt4 = pts.pop(wl)
                    uu = u_pool.tile([128, 2, W], qdt, tag="u", name="ulast")
                    for cc in range(2):
                        ta = g_pool.tile([128, W], qdt, tag="g", name=f"tal{cc}")
                        th2 = g_pool.tile([128, W], qdt, tag="g", name=f"thl{cc}")
                        nc.scalar.activation(out=ta, in_=pt4[:, cc, :], func=Tanh)
                        nc.scalar.activation(
                            out=th2, in_=pt4[:, cc, :], func=Tanh, scale=0.5
                        )
                        nc.vector.scalar_tensor_tensor(
                            out=uu[:, cc, :],
                            in0=th2,
                            scalar=1.0,
                            in1=ta,
                            op0=Add,
                            op1=Mult,
                        )
                    us[wl] = uu
                    outws[wl] = out_pool.tile([128, 4, W], qdt, tag="outs", name=f"outw{wl}")
                    psps = {}
                    for half in range(2):
                        ds = (0, 1) if half == 0 else (2, 3)
                        for cc in range(2):
                            for d in ds:
                                if cc == 0:
                                    psps[d] = work_pool.tile(
                                        [128, W], f32, tag="work", name=f"pspl{d}"
                                    )
                                nc.tensor.matmul(
                                    psps[d],
                                    wc_sb[:, cc, ts(d, 128)],
                                    uu[:, cc, :],
                                    start=(cc == 0),
                                    stop=(cc == 1),
                                )
                        for d in ds:
                            nc.vector.tensor_scalar_add(
                                outws[wl][:, d, :], psps[d], bb_sb[:, d : d + 1]
                            )
                        nc.sync.dma_start(
                            out=out_d[:, 2 * half : 2 * half + 2, ts(wl, W)],
                            in_=outws[wl][:, 2 * half : 2 * half + 2, :],
                        )

                if wp >= 0 and wp < NWIN - 2 and w is not None:
                    nc.sync.dma_start(out=out_d[:, :, ts(wp, W)], in_=outws[wp])

    nc.compile()
    return nc


def _get_program():
    if "nc" not in _CACHE:
        _CACHE["nc"] = _build_program()
    return _CACHE["nc"]


def _make_in_maps(x, Wr, br, Ws, bs):
    import ml_dtypes

    bf16 = ml_dtypes.bfloat16
    x = np.asarray(x, dtype=np.float32)
    Wr = np.asarray(Wr, dtype=np.float32)
    br = np.asarray(br, dtype=np.float32)
    Ws = np.asarray(Ws, dtype=np.float32)
    bs = np.asarray(bs, dtype=np.float32)

    # 0.5x from the sigmoid(a) = (1 + tanh(a/2))/2 identity folded into the
    # weights; res and skip fused along the output dim; stored c-major so the
    # weight chunks are the projection lhsT directly: wc[p, cc, d]
    wcomb = np.concatenate([0.5 * Wr, 0.5 * Ws], axis=0)  # [512 d, 256 c]
    wc = np.ascontiguousarray(
        wcomb.T.reshape(2, 128, 2 * C).transpose(1, 0, 2)
    )  # [128 p, 2 cc, 512 d]
    bcomb = np.concatenate([br, bs])  # [512]
    bb = np.ascontiguousarray(bcomb.reshape(4, 128).T)  # [128 p, 4 d]

    in_maps = []
    for i in range(NCORES):
        b, h = divmod(i, 2)
        xh = np.empty((TH, C), np.float32)
        if h == 0:
            xh[:W] = 0.0
            flag = np.zeros((128, 1), np.float32)
        else:
            xh[:W] = x[b, TCH - W : TCH]
            flag = np.ones((128, 1), np.float32)
        xh[W:] = x[b, h * TCH : (h + 1) * TCH]
        # xt[p, blk, cc, t] = xh[blk*512 + t, cc*128 + p]
        xt = xh.reshape(NBLK, W, 2, 128).transpose(3, 0, 2, 1)
        # xn[p, blk, s, c] = xh[blk*512 + s*128 + p, c], + [1, 0] tail
        xn = np.empty((128, NBLK, 4, C + 2), np.float32)
        xn[:, :, :, 0:C] = xh.reshape(NBLK, 4, 128, C).transpose(2, 0, 1, 3)
        xn[:, :, :, C] = 1.0
        xn[:, :, :, C + 1] = 0.0
        in_maps.append(
            {
                "xt": np.ascontiguousarray(xt.astype(bf16)),
                "xn": np.ascontiguousarray(xn.astype(bf16)),
                "wc": wc.astype(bf16),
                "bb": bb,
                "hflag": flag,
            }
        )
    return in_maps


def _gather(results):
    residual = np.empty((B, T, C), np.float32)
    skip = np.empty((B, T, C), np.float32)
    for i in range(NCORES):
        b, h = divmod(i, 2)
        o = results[i]["out"].astype(np.float32)  # [128 p, 4 d, 2048 t]
        rows = slice(h * TCH, (h + 1) * TCH)
        # res[t, c=dc*128+p] = o[p, dc, t]
        residual[b, rows] = o[:, 0:2, :].transpose(2, 1, 0).reshape(TCH, C)
        skip[b, rows] = o[:, 2:4, :].transpose(2, 1, 0).reshape(TCH, C)
    return residual, skip


def kernel(x, Wr, br, Ws, bs):
    from concourse.bass_utils import run_bass_kernel_spmd

    nc = _get_program()
    in_maps = _make_in_maps(x, Wr, br, Ws, bs)
    res = run_bass_kernel_spmd(nc, in_maps, list(range(NCORES)))
    return _gather(res.results)


# revision 15
# speedup vs baseline: 1.0450x; 1.0450x over previous
"""Trainium2 Bass kernel for nn_AttentionResBlock (windowed causal attention +
sigmoid*tanh gating + two 1x1 convs), SPMD over 8 NeuronCores.

Sharding: data-parallel over (batch, sequence-half): core i handles batch i//2,
rows [h*2048, (h+1)*2048) with h = i%2, plus a 512-row halo (previous window;
zeros + mask flag for h==0). No cross-core communication.

Layout strategy (v2): the host pre-computes BOTH layouts of x the PE needs —
xt (c-major, [128, 2cc, t]) for the QK/AV lhsT/rhs operands and xn (t-major,
[128, s, c+2] with a baked [1, 0] tail whose ones-column yields the softmax
denominator inside the AV matmul) — so the device does no input transposes at
all and every DMA lands with >=2KB contiguous per-partition segments.

Per-core pipeline (window = 512 queries, kv = 1024 keys):
  scoresT[j,q] = kvT^T @ qT        (PE bf16; jc5/jc6/jc7 trimmed to their
      causally-valid q ranges; garbage in merged tiles is exp'd then zeroed
      by the same affine_select that applies the causal mask)
  expT = exp(scale*scoresT)        (ACT, PSUM->SBUF, bf16)
  o_unnorm[q, c+2] = sum_j expT^T @ [v | 1 | 0]   (PE)
  o = o_unnorm * recip(denom)      (DVE per-partition scalar, PSUM drain)
  oT via PE transpose; gating u = tanh(a)*(1+tanh(a/2)) reads the transpose
      PSUM directly (ACT tanh x2 + one fused DVE scalar_tensor_tensor); the
      0.5 of sigmoid(a) = (1+tanh(a/2))/2 is folded into the weights.
  projT[d, t] = wc^T @ uT          (PE; res/skip fused along d = 4 chunks of
      128; bias is now PER-PARTITION so the PSUM drain is a single DVE
      tensor_scalar add, bf16 out; the host un-transposes the [d, t] output)

Schedule: software pipeline with a TWO-round lag on the output path: round r
runs QK/exp/AV/transpose for window r, the gating for window r-1 (emitted
after round r's exps so they feed AV first), and the projections for window
r-2 (whose inputs are a full round old — they interleave into round r's
exp-latency stalls without ever blocking the in-order PE queue). Junk
matmuls warm the HAM clock-gate during the initial DMA window. PSUM: 4 score
slots + 2 shared AV/proj slots + 2 transpose slots = exactly 8 banks.
"""

import numpy as np

B, T, C = 4, 4096, 256
W = 512                # attention window
TCH = T // 2           # rows per core
TH = TCH + W           # with halo
NWIN = TCH // W        # windows per core (4)
NBLK = TH // W         # 512-row blocks (5)
NCORES = 8

_CACHE = {}


def _build_program(qk_dtype_name="bfloat16"):
    import concourse.bacc as bacc
    import concourse.bass as bass
    import concourse.mybir as mybir
    import concourse.tile as tile
    from concourse.masks import make_identity

    f32 = mybir.dt.float32
    qdt = getattr(mybir.dt, qk_dtype_name)
    ts = bass.ts

    nc = bacc.Bacc("TRN2", target_bir_lowering=False, debug=False)

    xt_d = nc.dram_tensor("xt", [128, NBLK, 2, W], qdt, kind="ExternalInput").ap()
    xn_d = nc.dram_tensor("xn", [128, NBLK, 4, C + 2], qdt, kind="ExternalInput").ap()
    wc_d = nc.dram_tensor("wc", [128, 2, 2 * C], qdt, kind="ExternalInput").ap()
    hflag = nc.dram_tensor("hflag", [128, 1], f32, kind="ExternalInput").ap()
    out_d = nc.dram_tensor("out", [128, 4, TCH], qdt, kind="ExternalOutput").ap()

    Exp = mybir.ActivationFunctionType.Exp
    Tanh = mybir.ActivationFunctionType.Tanh
    Add = mybir.AluOpType.add
    Mult = mybir.AluOpType.mult

    with tile.TileContext(nc) as tc:
        with (
            tc.tile_pool(name="singles", bufs=1) as singles,
            tc.tile_pool(name="xt", bufs=1) as xt_pool,
            tc.tile_pool(name="xn", bufs=1) as xn_pool,
            tc.tile_pool(name="ex", bufs=9) as ex_pool,
            tc.tile_pool(name="on", bufs=6) as on_pool,
            tc.tile_pool(name="g", bufs=4) as g_pool,
            tc.tile_pool(name="u", bufs=2) as u_pool,
            tc.tile_pool(name="outs", bufs=3) as out_pool,
            tc.tile_pool(name="small", bufs=8) as small,
            tc.tile_pool(name="psc", bufs=4, space="PSUM") as sc_pool,
            tc.tile_pool(name="pwork", bufs=2, space="PSUM") as work_pool,
            tc.tile_pool(name="pt", bufs=2, space="PSUM") as pt_pool,
        ):
            # ---- input DMAs, all on the sync HWDGE queue, ordered by when
            # consumers need them (xt0/xt1 gate the first QK). Scalar queue
            # stays DMA-free so the ACT pipeline is never stalled behind a
            # descriptor-gen instruction.
            xtb = [None] * NBLK
            xnb = [None] * NBLK
            hf_sb = singles.tile([128, 1], f32)
            wc_sb = singles.tile([128, 2, 2 * C], qdt)

            # startup-critical loads ride BOTH HWDGE queues in parallel:
            # xt0 (sync) and xt1 (scalar) land together ~1.5us earlier than a
            # single combined transfer; everything later streams behind on
            # sync. The scalar queue is clear again before its ACT table
            # load + first exp.
            xt0 = xt_pool.tile([128, 2, W], qdt, tag="xt0")
            nc.sync.dma_start(out=xt0, in_=xt_d[:, 0, :, :])
            xt1 = xt_pool.tile([128, 2, W], qdt, tag="xt1")
            nc.scalar.dma_start(out=xt1, in_=xt_d[:, 1, :, :])
            xn01 = xn_pool.tile([128, 2, 4, C + 2], qdt, tag="xn01")
            nc.sync.dma_start(out=xn01, in_=xn_d[:, 0:2, :, :])
            nc.scalar.dma_start(out=wc_sb, in_=wc_d)
            nc.sync.dma_start(out=hf_sb, in_=hflag)
            xt234 = xt_pool.tile([128, 3, 2, W], qdt, tag="xt234")
            nc.sync.dma_start(out=xt234, in_=xt_d[:, 2:5, :, :])
            xn234 = xn_pool.tile([128, 3, 4, C + 2], qdt, tag="xn234")
            nc.sync.dma_start(out=xn234, in_=xn_d[:, 2:5, :, :])
            xtb[0], xtb[1] = xt0, xt1
            xnb[0] = xn01[:, 0]
            xnb[1] = xn01[:, 1]
            for blk in range(2, 5):
                xtb[blk] = xt234[:, blk - 2]
                xnb[blk] = xn234[:, blk - 2]

            # touch exp once so the ACT table set loads during the DMA shadow
            actwarm = small.tile([128, 1], f32, tag="rc")
            nc.vector.memset(actwarm, 0.0)
            nc.scalar.activation(out=actwarm, in_=actwarm, func=Exp)

            # HAM warmup: junk matmuls keep the PE activity window non-idle
            # from ~immediately after the NEFF barrier until the first real
            # QK, so the 4/8->8/8 clock-gate ramp happens during the DMA
            # shadow instead of eating the first windows' matmuls. Sized to
            # undershoot the DMA window (a short PE gap is harmless; junk
            # overrunning it would stall the first QK on the in-order queue).
            junk = singles.tile([128, 2 * C], qdt)
            nc.vector.memset(junk, 0.0)
            identf = singles.tile([128, 128], f32)
            make_identity(nc, identf)
            ident = singles.tile([128, 128], qdt)
            nc.vector.tensor_copy(ident, identf)
            for i in range(11):
                pwarm = work_pool.tile([128, 448], f32, tag="work")
                nc.tensor.matmul(
                    pwarm, junk[:, 0:128], junk[:, 0:448], start=True, stop=True
                )

            # ---------------- per-round emission helpers ----------------
            def qk_exp(w, jc):
                """QK matmuls + exp for one 128-row j-chunk; jc5 computes only
                its causally-reachable q range."""
                kvt = xtb[w + jc // 4]
                qt = xtb[w + 1]
                q_lo = 128 if jc == 5 else 0
                psc = sc_pool.tile([128, W - q_lo], f32, tag="sc", name=f"sc{jc}")
                for cc in range(2):
                    nc.tensor.matmul(
                        psc,
                        kvt[:, cc, ts(jc % 4, 128)],
                        qt[:, cc, q_lo:W],
                        start=(cc == 0),
                        stop=(cc == 1),
                    )
                ex = ex_pool.tile([128, W - q_lo], qdt, tag="ex", name=f"ex{jc}")
                nc.scalar.activation(out=ex, in_=psc, func=Exp, scale=0.0625)
                if jc == 4 or jc == 5:
                    # causal mask: valid iff q >= j - 512 (col is q - q_lo)
                    nc.gpsimd.affine_select(
                        out=ex,
                        in_=ex,
                        compare_op=mybir.AluOpType.is_ge,
                        fill=0.0,
                        base=0,
                        channel_multiplier=-1,
                        pattern=[[1, W - q_lo]],
                    )
                return (ex, q_lo)

            def qk_exp_67(w):
                """jc6 (full 256 q) and jc7 share one single-bank PSUM tile;
                the 3D causal affine_select masks both diagonals."""
                kvt = xtb[w + 1]
                qt = xtb[w + 1]
                psc = sc_pool.tile([128, 2, 256], f32, tag="sc", name="sc67")
                for i, jc in enumerate((6, 7)):
                    for cc in range(2):
                        nc.tensor.matmul(
                            psc[:, i, :],
                            kvt[:, cc, ts(jc % 4, 128)],
                            qt[:, cc, 256:W],
                            start=(i == 0 and cc == 0),
                            stop=(i == 1 and cc == 1),
                        )
                ex67 = ex_pool.tile([128, 2, 256], qdt, tag="ex", name="ex67")
                nc.scalar.activation(out=ex67, in_=psc, func=Exp, scale=0.0625)
                nc.gpsimd.affine_select(
                    out=ex67,
                    in_=ex67,
                    compare_op=mybir.AluOpType.is_ge,
                    fill=0.0,
                    base=0,
                    channel_multiplier=-1,
                    pattern=[[-128, 2], [1, 256]],
                )
                return ex67

            def av_qb(w, qb, expts):
                """one 128-query block of AV (+denominator) + normalize."""
                jcs = list(range(min(qb + 5, 8)))
                pav = work_pool.tile([128, C + 2], f32, tag="work")
                for k, jc in enumerate(jcs):
                    ap, q_lo = expts[jc]
                    nc.tensor.matmul(
                        pav,
                        ap[:, qb * 128 - q_lo : qb * 128 - q_lo + 128],
                        xnb[w + jc // 4][:, jc % 4, :],
                        start=(k == 0),
                        stop=(k == len(jcs) - 1),
                    )
                rc = small.tile([128, 1], f32, tag="rc")
                nc.vector.reciprocal(rc, pav[:, C : C + 1])
                on = on_pool.tile([128, C], qdt, tag="on")
                nc.vector.tensor_scalar_mul(on, pav[:, 0:C], rc)
                return on

            def proj_chunk(wp, d, u, outw, drain="v"):
                """one 128-channel output chunk of the fused res|skip
                projection, transposed: psp[d, t] = wc_d^T @ uT. The bias is
                applied host-side, so the PSUM drain is a pure copy — on DVE
                normally, or on the (tail-idle) ACT engine in the epilogue."""
                psp = work_pool.tile([128, W], f32, tag="work")
                for cc in range(2):
                    nc.tensor.matmul(
                        psp,
                        wc_sb[:, cc, ts(d, 128)],
                        u[:, cc, :],
                        start=(cc == 0),
                        stop=(cc == 1),
                    )
                if drain == "v":
                    nc.vector.tensor_copy(outw[:, d, :], psp)
                else:
                    nc.scalar.copy(outw[:, d, :], psp)

            pts = {}   # window -> oT transpose PSUM tile(s)
            us = {}    # window -> gated uT SBUF tile
            outws = {} # window -> output staging tile

            WL = NWIN - 1
            for r in range(NWIN + 1):
                w = r if r < NWIN else None
                wg = r - 1  # window whose gating runs this round
                wp = r - 2  # window whose (interleaved) projections run
                if wp >= 0 and wp < WL - 1:
                    outws[wp] = out_pool.tile([128, 4, W], qdt, tag="outs", name=f"outw{wp}")

                if w is not None:
                    # QK groups with wp's projections interleaved at the
                    # points where the PE would otherwise wait on exp slots
                    expts = [None] * 8
                    expts[0] = qk_exp(w, 0)
                    expts[1] = qk_exp(w, 1)
                    if wp >= 0:
                        proj_chunk(wp, 0, us[wp], outws[wp])
                    expts[2] = qk_exp(w, 2)
                    expts[3] = qk_exp(w, 3)
                    if wp >= 0:
                        proj_chunk(wp, 1, us[wp], outws[wp])
                    expts[4] = qk_exp(w, 4)
                    expts[5] = qk_exp(w, 5)
                    if wp >= 0:
                        proj_chunk(wp, 2, us[wp], outws[wp])
                    ex67 = qk_exp_67(w)
                    expts[6] = (ex67[:, 0, :], 256)
                    expts[7] = (ex67[:, 1, :], 256)
                    if w == 0:
                        # halo validity flag (1.0 = real halo, 0 = first win)
                        for jc in range(4):
                            nc.vector.tensor_scalar_mul(
                                expts[jc][0], expts[jc][0], hf_sb
                            )
                    ons = [av_qb(w, 0, expts)]
                    if wp >= 0:
                        proj_chunk(wp, 3, us[wp], outws[wp])
                    for qb in range(1, 4):
                        ons.append(av_qb(w, qb, expts))
                    if w < WL:
                        pt4 = pt_pool.tile([128, 2, W], qdt, tag="pt")
                        for qb in range(4):
                            for cc in range(2):
                                nc.tensor.transpose(
                                    pt4[:, cc, ts(qb, 128)],
                                    ons[qb][:, ts(cc, 128)],
                                    ident,
                                )
                        pts[w] = pt4
                    else:
                        # final window: transposes split into t-halves so the
                        # epilogue gating can start on half A while half B's
                        # AV is still in flight
                        ptA = pt_pool.tile([128, 2, 256], qdt, tag="pt", name="ptA")
                        ptB = pt_pool.tile([128, 2, 256], qdt, tag="pt", name="ptB")
                        for qb in range(4):
                            dst = ptA if qb < 2 else ptB
                            for cc in range(2):
                                nc.tensor.transpose(
                                    dst[:, cc, ts(qb % 2, 128)],
                                    ons[qb][:, ts(cc, 128)],
                                    ident,
                                )
                        pts[WL] = (ptA, ptB)

                if wg >= 0 and wg < WL:
                    # gating for window wg, emitted after round r's exps so
                    # those win the ACT queue; u = tanh(a) * (1 + tanh(a/2))
                    pt4 = pts.pop(wg)
                    ta = g_pool.tile([128, 2, W], qdt, tag="g")
                    th2 = g_pool.tile([128, 2, W], qdt, tag="g")
                    nc.scalar.activation(out=ta, in_=pt4, func=Tanh)
                    nc.scalar.activation(out=th2, in_=pt4, func=Tanh, scale=0.5)
                    uu = u_pool.tile([128, 2, W], qdt, tag="u")
                    nc.vector.scalar_tensor_tensor(
                        out=uu, in0=th2, scalar=1.0, in1=ta, op0=Add, op1=Mult
                    )
                    us[wg] = uu

                if w == WL:
                    # round 3 tail: window WL-1's projections run here (the
                    # PE is otherwise idle behind the epilogue's ACT chain);
                    # their drains ride DVE, the DMA follows.
                    wq = WL - 1
                    outws[wq] = out_pool.tile([128, 4, W], qdt, tag="outs", name=f"outw{wq}")
                    for d in range(4):
                        proj_chunk(wq, d, us[wq], outws[wq])
                    nc.sync.dma_start(out=out_d[:, :, ts(wq, W)], in_=outws[wq])

                if w is None:
                    # epilogue: final window's gating + projections pipelined
                    # per t-half; PSUM drains split across the idle ACT (A)
                    # and DVE (B) so neither serializes the tail.
                    ptA, ptB = pts.pop(WL)
                    uu = u_pool.tile([128, 2, W], qdt, tag="u", name="ulast")
                    outws[WL] = out_pool.tile([128, 4, W], qdt, tag="outs", name=f"outw{WL}")
                    for h, pth in enumerate((ptA, ptB)):
                        ta = g_pool.tile([128, 2, 256], qdt, tag="g", name=f"tal{h}")
                        th2 = g_pool.tile([128, 2, 256], qdt, tag="g", name=f"thl{h}")
                        nc.scalar.activation(out=ta, in_=pth, func=Tanh)
                        nc.scalar.activation(out=th2, in_=pth, func=Tanh, scale=0.5)
                        nc.vector.scalar_tensor_tensor(
                            out=uu[:, :, ts(h, 256)],
                            in0=th2,
                            scalar=1.0,
                            in1=ta,
                            op0=Add,
                            op1=Mult,
                        )
                    us[WL] = uu

                    def projl(d, h, psps):
                        psp = work_pool.tile([128, 256], f32, tag="work", name=f"pl{d}{h}")
                        for cc in range(2):
                            nc.tensor.matmul(
                                psp,
                                wc_sb[:, cc, ts(d, 128)],
                                uu[:, cc, ts(h, 256)],
                                start=(cc == 0),
                                stop=(cc == 1),
                            )
                        psps[(d, h)] = psp

                    psps = {}
                    for dpair in range(2):
                        d0, d1 = 2 * dpair, 2 * dpair + 1
                        projl(d0, 0, psps)
                        projl(d1, 0, psps)
                        nc.scalar.copy(outws[WL][:, d0, 0:256], psps[(d0, 0)])
                        nc.scalar.copy(outws[WL][:, d1, 0:256], psps[(d1, 0)])
                        projl(d0, 1, psps)
                        projl(d1, 1, psps)
                        nc.vector.tensor_copy(outws[WL][:, d0, 256:W], psps[(d0, 1)])
                        nc.vector.tensor_copy(outws[WL][:, d1, 256:W], psps[(d1, 1)])
                        nc.sync.dma_start(
                            out=out_d[:, d0 : d0 + 2, ts(WL, W)],
                            in_=outws[WL][:, d0 : d0 + 2, :],
                        )

                if wp >= 0 and wp < WL - 1 and w is not None:
                    nc.sync.dma_start(out=out_d[:, :, ts(wp, W)], in_=outws[wp])

    nc.compile()
    return nc


def _get_program():
    if "nc" not in _CACHE:
        _CACHE["nc"] = _build_program()
    return _CACHE["nc"]


def _make_in_maps(x, Wr, br, Ws, bs):
    import ml_dtypes

    bf16 = ml_dtypes.bfloat16
    x = np.asarray(x, dtype=np.float32)
    Wr = np.asarray(Wr, dtype=np.float32)
    br = np.asarray(br, dtype=np.float32)
    Ws = np.asarray(Ws, dtype=np.float32)
    bs = np.asarray(bs, dtype=np.float32)

    # 0.5x from the sigmoid(a) = (1 + tanh(a/2))/2 identity folded into the
    # weights; res and skip fused along the output dim; stored c-major so the
    # weight chunks are the projection lhsT directly: wc[p, cc, d]
    wcomb = np.concatenate([0.5 * Wr, 0.5 * Ws], axis=0)  # [512 d, 256 c]
    wc = np.ascontiguousarray(
        wcomb.T.reshape(2, 128, 2 * C).transpose(1, 0, 2)
    )  # [128 p, 2 cc, 512 d]
    in_maps = []
    for i in range(NCORES):
        b, h = divmod(i, 2)
        xh = np.empty((TH, C), np.float32)
        if h == 0:
            xh[:W] = 0.0
            flag = np.zeros((128, 1), np.float32)
        else:
            xh[:W] = x[b, TCH - W : TCH]
            flag = np.ones((128, 1), np.float32)
        xh[W:] = x[b, h * TCH : (h + 1) * TCH]
        # xt[p, blk, cc, t] = xh[blk*512 + t, cc*128 + p]
        xt = xh.reshape(NBLK, W, 2, 128).transpose(3, 0, 2, 1)
        # xn[p, blk, s, c] = xh[blk*512 + s*128 + p, c], + [1, 0] tail
        xn = np.empty((128, NBLK, 4, C + 2), np.float32)
        xn[:, :, :, 0:C] = xh.reshape(NBLK, 4, 128, C).transpose(2, 0, 1, 3)
        xn[:, :, :, C] = 1.0
        xn[:, :, :, C + 1] = 0.0
        in_maps.append(
            {
                "xt": np.ascontiguousarray(xt.astype(bf16)),
                "xn": np.ascontiguousarray(xn.astype(bf16)),
                "wc": wc.astype(bf16),
                "hflag": flag,
            }
        )
    return in_maps


def _gather(results, br, bs):
    residual = np.empty((B, T, C), np.float32)
    skip = np.empty((B, T, C), np.float32)
    for i in range(NCORES):
        b, h = divmod(i, 2)
        o = results[i]["out"].astype(np.float32)  # [128 p, 4 d, 2048 t]
        rows = slice(h * TCH, (h + 1) * TCH)
        # res[t, c=dc*128+p] = o[p, dc, t]; bias applied host-side
        residual[b, rows] = o[:, 0:2, :].transpose(2, 1, 0).reshape(TCH, C)
        skip[b, rows] = o[:, 2:4, :].transpose(2, 1, 0).reshape(TCH, C)
    residual += np.asarray(br, np.float32)[None, None, :]
    skip += np.asarray(bs, np.float32)[None, None, :]
    return residual, skip


def kernel(x, Wr, br, Ws, bs):
    from concourse.bass_utils import run_bass_kernel_spmd

    nc = _get_program()
    in_maps = _make_in_maps(x, Wr, br, Ws, bs)
    res = run_bass_kernel_spmd(nc, in_maps, list(range(NCORES)))
    return _gather(res.results, br, bs)
